# revision 44
# baseline (speedup 1.0000x reference)
"""HAN kernel for Trainium2, 8-core data parallel (4 docs/core)."""
import sys
sys.path.insert(0, "/opt/trn_rl_repo")
import numpy as np
import ml_dtypes

import concourse.bass as bass
import concourse.bacc as bacc
import concourse.mybir as mybir
import concourse.tile as tile
from concourse.bass_utils import run_bass_kernel_spmd

BF16 = mybir.dt.bfloat16
F32 = mybir.dt.float32
FP8 = mybir.dt.float8e4
I32 = mybir.dt.int32
AF = mybir.ActivationFunctionType
ALU = mybir.AluOpType
AX = mybir.AxisListType

B, S, L = 32, 16, 64
V, E, P, H = 32000, 300, 100, 256
POL = 3
NCORES = 8
ND = B // NCORES          # 4 docs per core
NS = ND * S               # 64 sentences per core
NT = NS * L               # 4096 word tokens per core
NTILE = NT // 128         # 32 token tiles
TBLK = 8                  # lstm steps per g_pre block
NBLK = L // TBLK          # 8 blocks

# gate permutation [i,f,g,o] -> [i,f,o,g]
GPERM = np.concatenate([np.arange(0, 512), np.arange(768, 1024), np.arange(512, 768)])


def _bf(x):
    return np.asarray(x, dtype=np.float32).astype(ml_dtypes.bfloat16)


def _ap(base, off_delta, dims):
    return bass.AP(tensor=base.tensor, offset=base.offset + off_delta, ap=dims)


def _build_nc():
    nc = bacc.Bacc("TRN2", target_bir_lowering=False, debug=False, num_devices=NCORES)

    def din(name, shape, dt):
        return nc.declare_dram_parameter(name, list(shape), dt, isOutput=False)

    t = {}
    t["tok_idx"] = din("tok_idx", [NTILE, 128, 1], I32)
    t["wp_idx"] = din("wp_idx", [NTILE, 128, 1], I32)
    t["sp_idx"] = din("sp_idx", [NS, 1], I32)
    t["emb_t"] = din("emb_t", [V, E], BF16)
    t["wpe_t"] = din("wpe_t", [1005, P], BF16)
    t["spe_t"] = din("spe_t", [25, P], BF16)
    t["wihT"] = din("wihT", [2, 4, 128, 1024], BF16)
    t["whhT"] = din("whhT", [2, 2, 128, 1024], BF16)
    t["swihT"] = din("swihT", [2, 5, 128, 1024], BF16)
    t["swhhT"] = din("swhhT", [2, 2, 128, 1024], BF16)
    t["wordW"] = din("wordW", [5, 128, 640], BF16)
    t["wordb"] = din("wordb", [128, 5], F32)
    t["wproj"] = din("wproj", [128, 5], BF16)
    t["sentW"] = din("sentW", [5, 128, 640], BF16)
    t["sentb"] = din("sentb", [128, 5], F32)
    t["sproj"] = din("sproj", [128, 5], BF16)
    t["denseWT"] = din("denseWT", [8, 128, POL], BF16)
    t["denseb"] = din("denseb", [POL, 1], F32)
    t["id128"] = din("id128", [128, 128], BF16)
    t["m2"] = din("m2", [128, 64], BF16)
    t["y_out"] = nc.declare_dram_parameter("y", [ND, POL], F32, isOutput=True)
    t["scores_d"] = nc.dram_tensor("scores_d", [NT], F32).ap()
    t["scs_d"] = nc.dram_tensor("scs_d", [NS], F32).ap()
    t["alps_d"] = nc.dram_tensor("alps_d", [ND, S], BF16).ap()

    with tile.TileContext(nc) as tc:
        from contextlib import ExitStack
        with ExitStack() as ctx:
            _emit(nc, tc, t, ctx)
    nc.compile()
    return nc


def _emit(nc, tc, t, ctx):
    wgt = ctx.enter_context(tc.tile_pool(name="wgt", bufs=1))
    big = ctx.enter_context(tc.tile_pool(name="big", bufs=1))
    gp = ctx.enter_context(tc.tile_pool(name="gp", bufs=2))
    gath = ctx.enter_context(tc.tile_pool(name="gath", bufs=4))
    small = ctx.enter_context(tc.tile_pool(name="small", bufs=3))
    st = ctx.enter_context(tc.tile_pool(name="st", bufs=1))
    amatp = ctx.enter_context(tc.tile_pool(name="amatp", bufs=1))
    ps = ctx.enter_context(tc.tile_pool(name="ps", bufs=2, space="PSUM"))
    psg = ctx.enter_context(tc.tile_pool(name="psg", bufs=2, space="PSUM"))
    pst = ctx.enter_context(tc.tile_pool(name="pst", bufs=2, space="PSUM"))

    # ---- constants / word weights ----
    ident = wgt.tile([128, 128], BF16)
    nc.sync.dma_start(out=ident[:], in_=t["id128"][:])
    m2_sb = wgt.tile([128, 64], BF16)
    nc.sync.dma_start(out=m2_sb[:], in_=t["m2"][:])

    # all gather indices in two DMAs (transposed load: [128, NTILE])
    ti_all = wgt.tile([128, NTILE], I32)
    wi_all = wgt.tile([128, NTILE], I32)
    nc.sync.dma_start(out=ti_all[:], in_=_ap(t["tok_idx"][:], 0, [[1, 128], [128, NTILE]]))
    nc.sync.dma_start(out=wi_all[:], in_=_ap(t["wp_idx"][:], 0, [[1, 128], [128, NTILE]]))

    wih_sb = wgt.tile([128, 2, 4, 1024], BF16, tag="wih")
    whh_sb = wgt.tile([128, 2, 2, 1024], BF16, tag="whh")
    for d in range(2):
        for k in range(4):
            nc.sync.dma_start(out=wih_sb[:, d, k, :], in_=t["wihT"][d, k])
        for k in range(2):
            nc.sync.dma_start(out=whh_sb[:, d, k, :], in_=t["whhT"][d, k])
    wordW_sb = wgt.tile([128, 5, 640], BF16, tag="attW")
    for k in range(5):
        nc.sync.dma_start(out=wordW_sb[:, k, :], in_=t["wordW"][k])
    wordb_sb = wgt.tile([128, 5], F32, tag="attb")
    nc.sync.dma_start(out=wordb_sb[:], in_=t["wordb"][:])
    wproj_sb = wgt.tile([128, 5], BF16, tag="attp")
    nc.sync.dma_start(out=wproj_sb[:], in_=t["wproj"][:])

    # ---- xT buffers; fills on the scalar engine (keeps gpsimd queue free) ----
    xT_emb = big.tile([128, 3, NT], BF16, tag="xTe")
    xT_pos = big.tile([128, NT], BF16, tag="xTp")
    nc.scalar.memzero(xT_emb[:, 2, :])
    nc.scalar.memzero(xT_pos[:])
    nc.scalar.add(xT_pos[:], xT_pos[:], 1.0)

    h_all = big.tile([128, 4, NT], BF16, tag="h_all")
    c_st = st.tile([128, 2, 2, 64], F32)

    hb_holder = {}

    def emit_htile(j):
        h_batch = hb_holder["t"]
        # h_batch[:, j] = transpose of h tile j (cols 128j..128j+127)
        for c in range(4):
            pt = pst.tile([128, 128], BF16, tag="pst", name=f"pt_h{j}_{c}")
            nc.tensor.transpose(out=pt[:], in_=h_all[:, c, j * 128:(j + 1) * 128],
                                identity=ident[:])
            if (j + c) % 2 == 0:
                nc.vector.tensor_copy(out=h_batch[:, j, c * 128:(c + 1) * 128],
                                      in_=pt[:])
            else:
                nc.scalar.copy(out=h_batch[:, j, c * 128:(c + 1) * 128], in_=pt[:])


    # ---- emission helpers (interleaved pipeline) ----
    def emit_gather(j):
        ge = gath.tile([128, E], BF16, tag="ge")
        gw = gath.tile([128, P], BF16, tag="gw")
        nc.gpsimd.indirect_dma_start(
            out=ge[:], out_offset=None, in_=t["emb_t"][:],
            in_offset=bass.IndirectOffsetOnAxis(ap=ti_all[:, j:j + 1], axis=0))
        nc.gpsimd.indirect_dma_start(
            out=gw[:], out_offset=None, in_=t["wpe_t"][:],
            in_offset=bass.IndirectOffsetOnAxis(ap=wi_all[:, j:j + 1], axis=0))
        cols = slice(j * 128, (j + 1) * 128)
        for c, (src, rows) in enumerate([(ge[:, 0:128], 128), (ge[:, 128:256], 128),
                                         (ge[:, 256:300], 44), (gw[:, :], 100)]):
            pt = pst.tile([128, 128], BF16, tag="pst")
            nc.tensor.transpose(out=pt[:rows, :], in_=src, identity=ident[:])
            dst = xT_pos[:rows, cols] if c == 3 else xT_emb[:rows, c, cols]
            if c % 2 == 0:
                nc.vector.tensor_copy(out=dst, in_=pt[:rows, :])
            else:
                nc.scalar.copy(out=dst, in_=pt[:rows, :])

    gpre_sb = {}

    def get_gpre(blk):
        if blk not in gpre_sb:
            gpre_sb[blk] = gp.tile([128, 2, 8, TBLK * 64], BF16, tag="gpre",
                                   name=f"gpre_b{blk}")
        return gpre_sb[blk]

    def emit_gpre(blk, d, m):
        gpre = get_gpre(blk)
        tok0 = blk * TBLK * 64 if d == 0 else (L - (blk + 1) * TBLK) * 64
        pgp = ps.tile([128, 512], F32, tag="ps")
        for k in range(4):
            rhs = xT_pos[:, tok0:tok0 + 512] if k == 3 else \
                xT_emb[:, k, tok0:tok0 + 512]
            nc.tensor.matmul(
                out=pgp[:], lhsT=wih_sb[:, d, k, m * 128:(m + 1) * 128],
                rhs=rhs, start=(k == 0), stop=(k == 3),
                skip_group_check=True)
        if m % 4 != 3:
            nc.vector.tensor_copy(out=gpre[:, d, m, :], in_=pgp[:])
        else:
            nc.scalar.copy(out=gpre[:, d, m, :], in_=pgp[:])

    # gpre free layout is [d:4096, m:512, col:1]; fwd reads col-offset tt*64,
    # bwd reads (TBLK-1-tt)*64 — fold both into one AP via an adjusted d-stride
    PP = 2 * 8 * TBLK * 64   # gpre partition pitch (free elems per partition)

    def gpre_ap(gpre, tt, nm):
        dstride = 8 * TBLK * 64 + (TBLK - 1 - 2 * tt) * 64
        return _ap(gpre[:, 0, 0, :], tt * 64,
                   [[PP, 128], [dstride, 2], [TBLK * 64, nm], [1, 64]])

    def emit_step(tau):
        blk, tt = tau // TBLK, tau % TBLK
        gpre = get_gpre(blk)
        sig = small.tile([128, 2, 8, 64], F32, tag="sig")
        tg = small.tile([128, 2, 2, 64], F32, tag="tg")
        if tau == 0:
            # no recurrent term: activations read g_pre from SBUF directly
            nc.scalar.activation(out=sig[:, :, 0:6, :], in_=gpre_ap(gpre, tt, 6),
                                 func=AF.Sigmoid)
            gsl = gpre_ap(gpre, tt, 8)
            tga = bass.AP(tensor=gsl.tensor, offset=gsl.offset + 6 * TBLK * 64,
                          ap=[[PP, 128], [gsl.ap[1][0], 2], [TBLK * 64, 2], [1, 64]])
            nc.scalar.activation(out=tg[:], in_=tga, func=AF.Tanh)
            gps = None
        if tau > 0:
            gps = psg.tile([128, 2, 8, 64], F32, tag="psg")
        m1 = small.tile([128, 2, 2, 64], F32, tag="m1")
        mm = small.tile([128, 2, 2, 64], F32, tag="mm")
        tc_ = small.tile([128, 2, 2, 64], F32, tag="tc_")
        # per-direction pipelining: d0's activations/elementwise chain runs
        # while d1's matmuls stream; the next step's d0 matmuls (which only
        # need h_d0) overlap d1's chain
        for d in range(2):
            if tau > 0:
                off = tt * 64 if d == 0 else (TBLK - 1 - tt) * 64
                # g_pre lands in PSUM via identity matmul: it has no h
                # dependency, so the PE runs it during the h(t-1) wait
                nc.tensor.matmul(
                    out=gps[:, d], lhsT=ident[:],
                    rhs=gpre[:, d, :, off:off + 64],
                    start=True, stop=False, skip_group_check=True)
                hcol = tau - 1 if d == 0 else L - tau
                for k in range(2):
                    for m in range(8):
                        nc.tensor.matmul(
                            out=gps[:, d, m],
                            lhsT=whh_sb[:, d, k, m * 128:(m + 1) * 128],
                            rhs=h_all[:, 2 * d + k, hcol * 64:(hcol + 1) * 64],
                            start=False, stop=(k == 1 and m == 7),
                            skip_group_check=True)
                nc.scalar.activation(out=sig[:, d, 0:6, :], in_=gps[:, d, 0:6, :],
                                     func=AF.Sigmoid)
                nc.scalar.activation(out=tg[:, d], in_=gps[:, d, 6:8, :],
                                     func=AF.Tanh)
            nc.vector.tensor_mul(out=m1[:, d], in0=sig[:, d, 0:2, :], in1=tg[:, d])
            if tau == 0:
                nc.vector.tensor_copy(out=c_st[:, d], in_=m1[:, d])
            else:
                nc.vector.tensor_mul(out=mm[:, d], in0=sig[:, d, 2:4, :],
                                     in1=c_st[:, d])
                nc.vector.tensor_add(out=c_st[:, d], in0=m1[:, d], in1=mm[:, d])
            nc.scalar.activation(out=tc_[:, d], in_=c_st[:, d], func=AF.Tanh)
            col = tau if d == 0 else L - 1 - tau
            nc.vector.tensor_mul(
                out=h_all[:, 2 * d:2 * d + 2, col * 64:(col + 1) * 64],
                in0=sig[:, d, 4:6, :], in1=tc_[:, d])

    # gather order per group g (fwd tiles first, then bwd tiles)
    def gorder(g):
        return [4 * g, 4 * g + 1, 4 * g + 2, 4 * g + 3,
                31 - 4 * g, 30 - 4 * g, 29 - 4 * g, 28 - 4 * g]

    # ---- word attention emission units (interleaved into late LSTM steps) ----
    uT_tiles = {}

    def emit_u_m(nb, m):
        if nb not in uT_tiles:
            uT_tiles[nb] = gp.tile([128, 5, 512], BF16, tag="uT",
                                   name=f"uT_nb{nb}")
        uT = uT_tiles[nb]
        cols = slice(nb * 512, (nb + 1) * 512)
        pu = ps.tile([128, 512], F32, tag="ps")
        for k in range(5):
            rhs = xT_pos[:, cols] if k == 4 else h_all[:, k, cols]
            nc.tensor.matmul(
                out=pu[:], lhsT=wordW_sb[:, k, m * 128:(m + 1) * 128],
                rhs=rhs, start=(k == 0), stop=(k == 4))
        nc.scalar.activation(out=uT[:, m, :], in_=pu[:], func=AF.Tanh,
                             bias=wordb_sb[:, m:m + 1])

    def emit_u_score(nb):
        uT = uT_tiles[nb]
        psc = pst.tile([1, 512], F32, tag="pst")
        for k in range(5):
            nc.tensor.matmul(out=psc[:], lhsT=wproj_sb[:, k:k + 1], rhs=uT[:, k, :],
                            start=(k == 0), stop=(k == 4), skip_group_check=True)
        scst = small.tile([1, 512], F32, tag="scst")
        nc.vector.tensor_copy(out=scst[:], in_=psc[:])
        nc.sync.dma_start(out=t["scores_d"][nb * 512:(nb + 1) * 512], in_=scst[:])
        nc.sync.dma_start(out=sc2[0:64, 8 * nb:8 * nb + 8],
                          in_=_ap(t["scores_d"], nb * 512, [[1, 64], [64, 8]]))

    sc2 = st.tile([64, 64], F32)

    # flat unit queue for nb blocks whose h columns are complete mid-LSTM
    uq = []
    for nb in (3, 4, 2, 5, 1, 6):
        uq += [(nb, m) for m in range(5)] + [(nb, None)]

    # ---- pipelined word phase ----
    # fwd tiles first so d=0 g_pre can start after only 4 gathers
    g0 = gorder(0)
    for j in g0[:4]:
        emit_gather(j)
    for m in range(8):
        emit_gpre(0, 0, m)
    for j in g0[4:]:
        emit_gather(j)
    for m in range(8):
        emit_gpre(0, 1, m)
    uqi = 0
    for blk in range(NBLK):
        # groups 0-3 cover all 32 tiles (group g's bwd tiles = group 7-g's
        # fwd tiles), so only gather during blocks 0-2
        nxt = gorder(blk + 1) if blk + 1 <= 3 else None
        for tt in range(TBLK):
            tau = blk * TBLK + tt
            if nxt is not None and tt < 4:
                emit_gather(nxt[2 * tt])
                emit_gather(nxt[2 * tt + 1])
            if blk + 1 < NBLK and tt >= 4:
                i = tt - 4
                for q in range(4):
                    u = 4 * i + q
                    emit_gpre(blk + 1, u // 8, u % 8)
            emit_step(tau)
            # attention u-blocks for already-finished h columns
            if tau >= 48:
                budget = 2 if tau < 56 else 3
                for _ in range(budget):
                    if uqi < len(uq):
                        nb, m = uq[uqi]
                        uqi += 1
                        if m is None:
                            emit_u_score(nb)
                        else:
                            emit_u_m(nb, m)

    while uqi < len(uq):
        nb, m = uq[uqi]
        uqi += 1
        emit_u_score(nb) if m is None else emit_u_m(nb, m)
    for nb in (0, 7):
        for m in range(5):
            emit_u_m(nb, m)
        emit_u_score(nb)

    mx = small.tile([64, 1], F32, tag="mx")
    nc.vector.tensor_reduce(out=mx[:], in_=sc2[:], axis=AX.X, op=ALU.max, negate=True)
    ex = small.tile([64, 64], F32, tag="ex")
    den = small.tile([64, 1], F32, tag="den")
    nc.scalar.activation(out=ex[:], in_=sc2[:], func=AF.Exp, bias=mx[:],
                         accum_out=den[:])
    rcp = small.tile([64, 1], F32, tag="rcp")
    nc.vector.reciprocal(out=rcp[:], in_=den[:])
    alpha_bf = small.tile([64, 64], BF16, tag="alpha_bf")
    nc.vector.tensor_scalar_mul(out=alpha_bf[:], in0=ex[:], scalar1=rcp[:])
    alpha2 = small.tile([128, 64], BF16, tag="alpha2")
    nc.sync.dma_start(out=alpha2[0:64, :], in_=alpha_bf[:])
    nc.sync.dma_start(out=alpha2[64:128, :], in_=alpha_bf[:])
    # amat[p, j, s] = alpha[p%64, 2j + (p>=64)] * (p%64 == s)
    amat = amatp.tile([128, 32, 64], BF16, tag="amat")
    for half, eng in ((0, nc.vector), (1, nc.gpsimd)):
        pr = slice(half * 64, half * 64 + 64)
        eng.tensor_tensor(
            out=amat[pr, :, :],
            in0=_ap(m2_sb[pr, :], 0, [[64, 64], [0, 32], [1, 64]]),
            in1=_ap(alpha2[pr, :], half, [[64, 64], [2, 32], [0, 64]]),
            op=ALU.mult)
    hb_holder["t"] = big.tile([128, 32, 512], BF16, tag="xTe",
                              name="h_batch")
    h_batch = hb_holder["t"]
    for j in range(NTILE):
        emit_htile(j)
    psen = ps.tile([64, 512], F32, tag="ps")
    for j in range(NTILE):
        nc.tensor.matmul(out=psen[:], lhsT=amat[:, j, :], rhs=h_batch[:, j, :],
                        start=(j == 0), stop=(j == NTILE - 1),
                        skip_group_check=True)
    sen_sb = st.tile([64, 512], BF16)
    nc.vector.tensor_copy(out=sen_sb[:], in_=psen[:])

    # ---- sentence-level weights (reuse word slots) ----
    swih_sb = wgt.tile([128, 2, 5, 1024], BF16, tag="wih")
    swhh_sb = wgt.tile([128, 2, 2, 1024], BF16, tag="whh")
    for d in range(2):
        for k in range(5):
            nc.sync.dma_start(out=swih_sb[:, d, k, :], in_=t["swihT"][d, k])
        for k in range(2):
            nc.sync.dma_start(out=swhh_sb[:, d, k, :], in_=t["swhhT"][d, k])
    sentW_sb = wgt.tile([128, 5, 640], BF16, tag="attW")
    for k in range(5):
        nc.sync.dma_start(out=sentW_sb[:, k, :], in_=t["sentW"][k])
    sentb_sb = wgt.tile([128, 5], F32, tag="attb")
    nc.sync.dma_start(out=sentb_sb[:], in_=t["sentb"][:])
    sproj_sb = wgt.tile([128, 5], BF16, tag="attp")
    nc.sync.dma_start(out=sproj_sb[:], in_=t["sproj"][:])
    dW_sb = wgt.tile([128, 8, POL], BF16)
    for k in range(8):
        nc.sync.dma_start(out=dW_sb[:, k, :], in_=t["denseWT"][k])
    db_sb = wgt.tile([POL, 1], F32)
    nc.sync.dma_start(out=db_sb[:], in_=t["denseb"][:])

    # sxT [128, 5, 64] doc-major cols (d*16 + sigma)
    sxT = st.tile([128, 5, NS], BF16)
    nc.gpsimd.memset(sxT[:, 4, :], 0.0)
    nc.gpsimd.memset(sxT[96:128, 4, :], 1.0)
    si = gath.tile([NS, 1], I32, tag="ti")
    nc.sync.dma_start(out=si[:], in_=t["sp_idx"][:])
    gs = gath.tile([NS, P], BF16, tag="gw")
    nc.gpsimd.indirect_dma_start(
        out=gs[:], out_offset=None, in_=t["spe_t"][:],
        in_offset=bass.IndirectOffsetOnAxis(ap=si[:, :1], axis=0))
    pt_s = pst.tile([128, 64], BF16, tag="pst")
    nc.tensor.transpose(out=pt_s[:P, :], in_=gs[:], identity=ident[:64, :64])
    nc.vector.tensor_copy(out=sxT[:P, 4, :], in_=pt_s[:P, :])
    for c in range(4):
        ptv = pst.tile([128, 64], BF16, tag="pst")
        nc.tensor.transpose(out=ptv[:], in_=sen_sb[:, c * 128:(c + 1) * 128],
                            identity=ident[:64, :64])
        nc.vector.tensor_copy(out=sxT[:, c, :], in_=ptv[:])

    # sentence g_pre (all 16 steps at once)
    sgpre = st.tile([128, 2, 8, NS], BF16)
    for d in range(2):
        for m in range(8):
            pgs = ps.tile([128, 64], F32, tag="ps")
            for k in range(5):
                nc.tensor.matmul(
                    out=pgs[:], lhsT=swih_sb[:, d, k, m * 128:(m + 1) * 128],
                    rhs=sxT[:, k, :], start=(k == 0), stop=(k == 4))
            if m % 4 != 3:
                nc.vector.tensor_copy(out=sgpre[:, d, m, :], in_=pgs[:])
            else:
                nc.scalar.copy(out=sgpre[:, d, m, :], in_=pgs[:])

    # sentence BiLSTM (T=16, batch=4/dir), doc-major cols d*16+pos
    # tau=0: activations read sgpre directly; tau>0: whh matmul into PSUM,
    # then vector-add of the sgpre slice (no identity-extract matmul).
    hs_all = st.tile([128, 4, NS], BF16)
    cs_st = st.tile([128, 2, 2, ND], F32)
    for tau in range(S):
        if tau == 0:
            pos_f, pos_b = 0, S - 1
            src_f = _ap(sgpre[:, 0, 0, :], pos_f, [[1024, 128], [64, 8], [16, ND]])
            src_b = _ap(sgpre[:, 1, 0, :], pos_b, [[1024, 128], [64, 8], [16, ND]])
            sig = small.tile([128, 2, 8, ND], F32, tag="ssig")
            tg = small.tile([128, 2, 2, ND], F32, tag="stg")
            nc.scalar.activation(
                out=sig[:, 0, 0:6, :], func=AF.Sigmoid,
                in_=_ap(sgpre[:, 0, 0, :], pos_f, [[1024, 128], [64, 6], [16, ND]]))
            nc.scalar.activation(
                out=sig[:, 1, 0:6, :], func=AF.Sigmoid,
                in_=_ap(sgpre[:, 1, 0, :], pos_b, [[1024, 128], [64, 6], [16, ND]]))
            nc.scalar.activation(
                out=tg[:, 0], func=AF.Tanh,
                in_=_ap(sgpre[:, 0, 6, :], pos_f, [[1024, 128], [64, 2], [16, ND]]))
            nc.scalar.activation(
                out=tg[:, 1], func=AF.Tanh,
                in_=_ap(sgpre[:, 1, 6, :], pos_b, [[1024, 128], [64, 2], [16, ND]]))
        else:
            gps = psg.tile([128, 2, 8, 64], F32, tag="psg")
            sig = small.tile([128, 2, 8, ND], F32, tag="ssig")
            tg = small.tile([128, 2, 2, ND], F32, tag="stg")
        m1 = small.tile([128, 2, 2, ND], F32, tag="sm1")
        mm = small.tile([128, 2, 2, ND], F32, tag="smm")
        tc_ = small.tile([128, 2, 2, ND], F32, tag="stc")
        for d in range(2):
            if tau > 0:
                pos = tau if d == 0 else S - 1 - tau
                nc.tensor.matmul(
                    out=_ap(gps[:, d, 0, :], 0, [[1024, 128], [64, 8], [1, ND]]),
                    lhsT=ident[:],
                    rhs=_ap(sgpre[:, d, 0, :], pos, [[1024, 128], [64, 8], [16, ND]]),
                    start=True, stop=False, skip_group_check=True)
                hpos = tau - 1 if d == 0 else S - tau
                for k in range(2):
                    for m in range(8):
                        nc.tensor.matmul(
                            out=gps[:, d, m, 0:ND],
                            lhsT=swhh_sb[:, d, k, m * 128:(m + 1) * 128],
                            rhs=_ap(hs_all[:, 2 * d + k, :], hpos,
                                    [[256, 128], [16, ND]]),
                            start=False, stop=(k == 1 and m == 7),
                            skip_group_check=True)
                nc.scalar.activation(out=sig[:, d, 0:6, :],
                                     in_=gps[:, d, 0:6, 0:ND], func=AF.Sigmoid)
                nc.scalar.activation(out=tg[:, d], in_=gps[:, d, 6:8, 0:ND],
                                     func=AF.Tanh)
            nc.vector.tensor_mul(out=m1[:, d], in0=sig[:, d, 0:2, :], in1=tg[:, d])
            if tau == 0:
                nc.vector.tensor_copy(out=cs_st[:, d], in_=m1[:, d])
            else:
                nc.vector.tensor_mul(out=mm[:, d], in0=sig[:, d, 2:4, :],
                                     in1=cs_st[:, d])
                nc.vector.tensor_add(out=cs_st[:, d], in0=m1[:, d], in1=mm[:, d])
            nc.scalar.activation(out=tc_[:, d], in_=cs_st[:, d], func=AF.Tanh)
            pos = tau if d == 0 else S - 1 - tau
            nc.vector.tensor_mul(
                out=_ap(hs_all[:, 2 * d, :], pos, [[256, 128], [64, 2], [16, ND]]),
                in0=sig[:, d, 4:6, :], in1=tc_[:, d])

    # ---- sentence attention ----
    usT = st.tile([128, 5, NS], BF16)
    for m in range(5):
        pu = ps.tile([128, 64], F32, tag="ps")
        for k in range(5):
            rhs = sxT[:, 4, :] if k == 4 else hs_all[:, k, :]
            nc.tensor.matmul(out=pu[:], lhsT=sentW_sb[:, k, m * 128:(m + 1) * 128],
                            rhs=rhs, start=(k == 0), stop=(k == 4))
        nc.scalar.activation(out=usT[:, m, :], in_=pu[:], func=AF.Tanh,
                             bias=sentb_sb[:, m:m + 1])
    pscs = pst.tile([1, NS], F32, tag="pst")
    for k in range(5):
        nc.tensor.matmul(out=pscs[:], lhsT=sproj_sb[:, k:k + 1], rhs=usT[:, k, :],
                        start=(k == 0), stop=(k == 4))
    scs = small.tile([1, NS], F32, tag="scs")
    nc.vector.tensor_copy(out=scs[:], in_=pscs[:])
    sc2s = small.tile([ND, S], F32, tag="sc2s")
    nc.sync.dma_start(out=sc2s[:], in_=scs[0:1, 0:NS], single_packet=True)
    mxs = small.tile([ND, 1], F32, tag="mxs")
    nc.vector.tensor_reduce(out=mxs[:], in_=sc2s[:], axis=AX.X, op=ALU.max,
                            negate=True)
    exs = small.tile([ND, S], F32, tag="exs")
    dens = small.tile([ND, 1], F32, tag="dens")
    nc.scalar.activation(out=exs[:], in_=sc2s[:], func=AF.Exp, bias=mxs[:],
                         accum_out=dens[:])
    rcs = small.tile([ND, 1], F32, tag="rcs")
    nc.vector.reciprocal(out=rcs[:], in_=dens[:])
    alps_bf = small.tile([ND, S], BF16, tag="alps_bf")
    nc.vector.tensor_scalar_mul(out=alps_bf[:], in0=exs[:], scalar1=rcs[:])
    # amats [64, 4]: column d rows d*16:(d+1)*16 = alps_bf[d, :]
    amats = small.tile([64, ND], BF16, tag="amats")
    nc.vector.memset(amats[:], 0.0)
    for d in range(ND):
        eng = nc.sync if d % 2 == 0 else nc.scalar
        eng.dma_start(out=amats[d * S:(d + 1) * S, d:d + 1],
                      in_=alps_bf[d:d + 1, :], single_packet=True)
    hbs = st.tile([64, 512], BF16)
    for c in range(4):
        ptb = pst.tile([64, 128], BF16, tag="pst")
        nc.tensor.transpose(out=ptb[:], in_=hs_all[:, c, :], identity=ident[:])
        nc.vector.tensor_copy(out=hbs[:, c * 128:(c + 1) * 128], in_=ptb[:])
    # feats^T [128, 8, ND]: doc_vec computed directly in transposed layout
    featsT = st.tile([128, 8, ND], BF16)
    pdt = ps.tile([128, 4, ND], F32, tag="ps")
    for c in range(4):
        nc.tensor.matmul(out=pdt[:, c, :], lhsT=hbs[:, c * 128:(c + 1) * 128],
                         rhs=amats[:], start=(c == 0), stop=(c == 3),
                         skip_group_check=True)
    nc.vector.tensor_copy(out=featsT[:, 0:4, :], in_=pdt[:])
    nc.vector.tensor_copy(
        out=featsT[:, 4:6, :],
        in_=_ap(hs_all[:, 0, :], S - 1, [[256, 128], [64, 2], [16, ND]]))
    nc.vector.tensor_copy(
        out=featsT[:, 6:8, :],
        in_=_ap(hs_all[:, 2, :], 0, [[256, 128], [64, 2], [16, ND]]))
    pout = pst.tile([POL, ND], F32, tag="pst")
    for k in range(8):
        nc.tensor.matmul(out=pout[:], lhsT=dW_sb[:, k, :], rhs=featsT[:, k, :],
                        start=(k == 0), stop=(k == 7))
    yf = small.tile([POL, ND], F32, tag="yf")
    nc.scalar.activation(out=yf[:], in_=pout[:], func=AF.Identity, bias=db_sb[:])
    nc.sync.dma_start(out=_ap(t["y_out"][:], 0, [[1, POL], [POL, ND]]), in_=yf[:],
                      single_packet=True)


_NC_CACHE = None


def _get_nc():
    global _NC_CACHE
    if _NC_CACHE is None:
        _NC_CACHE = _build_nc()
    return _NC_CACHE


def _prep_host(inputs):
    gpm = GPERM
    wihs, whhs = [], []
    for wih, b in [(inputs["wWih_f"], inputs["wb_f"]),
                   (inputs["wWih_b"], inputs["wb_b"])]:
        wt = np.zeros((512, 1024), np.float32)
        wp = np.asarray(wih, np.float32)[gpm]      # [1024, 400]
        wt[0:300, :] = wp[:, 0:300].T
        wt[384:484, :] = wp[:, 300:400].T
        wt[511, :] = np.asarray(b, np.float32)[gpm]
        wihs.append(wt.reshape(4, 128, 1024))
    for whh in [inputs["wWhh_f"], inputs["wWhh_b"]]:
        whhs.append(np.asarray(whh, np.float32)[gpm].T.reshape(2, 128, 1024))
    swihs, swhhs = [], []
    for wih, b in [(inputs["sWih_f"], inputs["sb_f"]),
                   (inputs["sWih_b"], inputs["sb_b"])]:
        wt = np.zeros((640, 1024), np.float32)
        wp = np.asarray(wih, np.float32)[gpm]      # [1024, 612]
        wt[0:612, :] = wp.T
        wt[639, :] = np.asarray(b, np.float32)[gpm]
        swihs.append(wt.reshape(5, 128, 1024))
    for whh in [inputs["sWhh_f"], inputs["sWhh_b"]]:
        swhhs.append(np.asarray(whh, np.float32)[gpm].T.reshape(2, 128, 1024))

    def padW(w):
        o = np.zeros((640, 640), np.float32)
        o[:612, :612] = np.asarray(w, np.float32)
        return o.reshape(5, 128, 640)

    def padv(v, chunks):
        o = np.zeros((chunks * 128,), np.float32)
        o[:len(v)] = np.asarray(v, np.float32)
        return np.ascontiguousarray(o.reshape(chunks, 128).T)

    m2 = np.zeros((128, 64), np.float32)
    for p in range(128):
        m2[p, p % 64] = 1.0

    shared = {
        "emb_t": _bf(inputs["emb"]),
        "wpe_t": _bf(inputs["wpos_emb"]),
        "spe_t": _bf(inputs["spos_emb"]),
        "wihT": _bf(np.stack(wihs)),
        "whhT": _bf(np.stack(whhs)),
        "swihT": _bf(np.stack(swihs)),
        "swhhT": _bf(np.stack(swhhs)),
        "wordW": _bf(padW(inputs["word_W"])),
        "wordb": padv(inputs["word_bias"], 5).astype(np.float32),
        "wproj": _bf(padv(inputs["word_proj"], 5)),
        "sentW": _bf(padW(inputs["sent_W"])),
        "sentb": padv(inputs["sent_bias"], 5).astype(np.float32),
        "sproj": _bf(padv(inputs["sent_proj"], 5)),
        "denseWT": _bf(np.ascontiguousarray(
            np.asarray(inputs["dense_W"], np.float32).T).reshape(8, 128, POL)),
        "denseb": np.asarray(inputs["dense_b"], np.float32).reshape(POL, 1),
        "id128": _bf(np.eye(128)),
        "m2": _bf(m2),
    }

    toks = np.asarray(inputs["text_raw_indices"], np.int64).reshape(B, S, L)
    wpos = np.asarray(inputs["word_position"], np.int64).reshape(B, S, L)
    spos = np.asarray(inputs["segment_position"], np.int64).reshape(B, S)
    in_maps = []
    for c in range(NCORES):
        tk = toks[c * ND:(c + 1) * ND].reshape(NS, L)   # [64 s, 64 t]
        wp = wpos[c * ND:(c + 1) * ND].reshape(NS, L)
        sp = spos[c * ND:(c + 1) * ND]                  # [4, 16]
        m = dict(shared)
        m["tok_idx"] = np.ascontiguousarray(tk.T).reshape(NTILE, 128, 1).astype(np.int32)
        m["wp_idx"] = np.ascontiguousarray(wp.T).reshape(NTILE, 128, 1).astype(np.int32)
        m["sp_idx"] = sp.reshape(NS, 1).astype(np.int32)
        in_maps.append(m)
    return in_maps


def kernel(**inputs):
    nc = _get_nc()
    in_maps = _prep_host(inputs)
    res = run_bass_kernel_spmd(nc, in_maps, list(range(NCORES)))
    out = np.concatenate([res.results[c]["y"] for c in range(NCORES)], axis=0)
    return out.astype(np.float32)


# revision 46
# speedup vs baseline: 1.1936x; 1.1936x over previous
"""HAN kernel for Trainium2, 8-core data parallel (4 docs/core)."""
import sys
sys.path.insert(0, "/opt/trn_rl_repo")
import numpy as np
import ml_dtypes

import concourse.bass as bass
import concourse.bacc as bacc
import concourse.mybir as mybir
import concourse.tile as tile
from concourse.bass_utils import run_bass_kernel_spmd

BF16 = mybir.dt.bfloat16
F32 = mybir.dt.float32
FP8 = mybir.dt.float8e4
I32 = mybir.dt.int32
AF = mybir.ActivationFunctionType
ALU = mybir.AluOpType
AX = mybir.AxisListType

B, S, L = 32, 16, 64
V, E, P, H = 32000, 300, 100, 256
POL = 3
NCORES = 8
ND = B // NCORES          # 4 docs per core
NS = ND * S               # 64 sentences per core
NT = NS * L               # 4096 word tokens per core
NTILE = NT // 128         # 32 token tiles
TBLK = 8                  # lstm steps per g_pre block
NBLK = L // TBLK          # 8 blocks

# gate permutation [i,f,g,o] -> [i,f,o,g]
GPERM = np.concatenate([np.arange(0, 512), np.arange(768, 1024), np.arange(512, 768)])


def _bf(x):
    return np.asarray(x, dtype=np.float32).astype(ml_dtypes.bfloat16)


def _ap(base, off_delta, dims):
    return bass.AP(tensor=base.tensor, offset=base.offset + off_delta, ap=dims)


def _build_nc():
    nc = bacc.Bacc("TRN2", target_bir_lowering=False, debug=False, num_devices=NCORES)

    def din(name, shape, dt):
        return nc.declare_dram_parameter(name, list(shape), dt, isOutput=False)

    t = {}
    t["tok_idx"] = din("tok_idx", [NTILE, 128, 1], I32)
    t["wp_idx"] = din("wp_idx", [NTILE, 128, 1], I32)
    t["sp_idx"] = din("sp_idx", [NS, 1], I32)
    t["emb_t"] = din("emb_t", [V, E], BF16)
    t["wpe_t"] = din("wpe_t", [1005, P], BF16)
    t["spe_t"] = din("spe_t", [25, P], BF16)
    t["wihT"] = din("wihT", [2, 4, 128, 1024], BF16)
    t["whhT"] = din("whhT", [2, 2, 128, 1024], BF16)
    t["swihT"] = din("swihT", [2, 5, 128, 1024], BF16)
    t["swhhT"] = din("swhhT", [2, 2, 128, 1024], BF16)
    t["wordW"] = din("wordW", [5, 128, 640], BF16)
    t["wordb"] = din("wordb", [128, 5], F32)
    t["wproj"] = din("wproj", [128, 5], BF16)
    t["sentW"] = din("sentW", [5, 128, 640], BF16)
    t["sentb"] = din("sentb", [128, 5], F32)
    t["sproj"] = din("sproj", [128, 5], BF16)
    t["denseWT"] = din("denseWT", [8, 128, POL], BF16)
    t["denseb"] = din("denseb", [POL, 1], F32)
    t["id128"] = din("id128", [128, 128], BF16)
    t["m2"] = din("m2", [128, 64], BF16)
    t["y_out"] = nc.declare_dram_parameter("y", [ND, POL], F32, isOutput=True)
    t["scores_d"] = nc.dram_tensor("scores_d", [NT], F32).ap()
    t["scs_d"] = nc.dram_tensor("scs_d", [NS], F32).ap()
    t["alps_d"] = nc.dram_tensor("alps_d", [ND, S], BF16).ap()

    with tile.TileContext(nc) as tc:
        from contextlib import ExitStack
        with ExitStack() as ctx:
            _emit(nc, tc, t, ctx)
    nc.compile()
    return nc


def _emit(nc, tc, t, ctx):
    wgt = ctx.enter_context(tc.tile_pool(name="wgt", bufs=1))
    big = ctx.enter_context(tc.tile_pool(name="big", bufs=1))
    gp = ctx.enter_context(tc.tile_pool(name="gp", bufs=2))
    gath = ctx.enter_context(tc.tile_pool(name="gath", bufs=4))
    small = ctx.enter_context(tc.tile_pool(name="small", bufs=3))
    st = ctx.enter_context(tc.tile_pool(name="st", bufs=1))
    amatp = ctx.enter_context(tc.tile_pool(name="amatp", bufs=1))
    ps = ctx.enter_context(tc.tile_pool(name="ps", bufs=2, space="PSUM"))
    psg = ctx.enter_context(tc.tile_pool(name="psg", bufs=2, space="PSUM"))
    pst = ctx.enter_context(tc.tile_pool(name="pst", bufs=2, space="PSUM"))

    # ---- constants / word weights ----
    ident = wgt.tile([128, 128], BF16)
    nc.sync.dma_start(out=ident[:], in_=t["id128"][:])
    m2_sb = wgt.tile([128, 64], BF16)
    nc.sync.dma_start(out=m2_sb[:], in_=t["m2"][:])

    # all gather indices in two DMAs (transposed load: [128, NTILE])
    ti_all = wgt.tile([128, NTILE], I32)
    wi_all = wgt.tile([128, NTILE], I32)
    nc.sync.dma_start(out=ti_all[:], in_=_ap(t["tok_idx"][:], 0, [[1, 128], [128, NTILE]]))
    nc.sync.dma_start(out=wi_all[:], in_=_ap(t["wp_idx"][:], 0, [[1, 128], [128, NTILE]]))

    wih_sb = wgt.tile([128, 2, 4, 1024], BF16, tag="wih")
    whh_sb = wgt.tile([128, 2, 2, 1024], BF16, tag="whh")
    for d in range(2):
        for k in range(4):
            nc.sync.dma_start(out=wih_sb[:, d, k, :], in_=t["wihT"][d, k])
        for k in range(2):
            nc.sync.dma_start(out=whh_sb[:, d, k, :], in_=t["whhT"][d, k])
    wordW_sb = wgt.tile([128, 5, 640], BF16, tag="attW")
    for k in range(5):
        nc.sync.dma_start(out=wordW_sb[:, k, :], in_=t["wordW"][k])
    wordb_sb = wgt.tile([128, 5], F32, tag="attb")
    nc.sync.dma_start(out=wordb_sb[:], in_=t["wordb"][:])
    wproj_sb = wgt.tile([128, 5], BF16, tag="attp")
    nc.sync.dma_start(out=wproj_sb[:], in_=t["wproj"][:])

    # ---- xT buffers; fills on the scalar engine (keeps gpsimd queue free) ----
    xT_emb = big.tile([128, 3, NT], BF16, tag="xTe")
    xT_pos = big.tile([128, NT], BF16, tag="xTp")
    nc.scalar.memzero(xT_emb[:, 2, :])
    nc.scalar.memzero(xT_pos[:])
    nc.scalar.add(xT_pos[:], xT_pos[:], 1.0)

    h_all = big.tile([128, 4, NT], BF16, tag="h_all")
    c_st = st.tile([128, 2, 2, 64], F32)

    hb_holder = {}

    def emit_htile(j):
        h_batch = hb_holder["t"]
        # h_batch[:, j] = transpose of h tile j (cols 128j..128j+127)
        for c in range(4):
            pt = pst.tile([128, 128], BF16, tag="pst", name=f"pt_h{j}_{c}")
            nc.tensor.transpose(out=pt[:], in_=h_all[:, c, j * 128:(j + 1) * 128],
                                identity=ident[:])
            if (j + c) % 2 == 0:
                nc.vector.tensor_copy(out=h_batch[:, j, c * 128:(c + 1) * 128],
                                      in_=pt[:])
            else:
                nc.scalar.copy(out=h_batch[:, j, c * 128:(c + 1) * 128], in_=pt[:])


    # ---- emission helpers (interleaved pipeline) ----
    def emit_gather(j):
        ge = gath.tile([128, E], BF16, tag="ge")
        gw = gath.tile([128, P], BF16, tag="gw")
        nc.gpsimd.indirect_dma_start(
            out=ge[:], out_offset=None, in_=t["emb_t"][:],
            in_offset=bass.IndirectOffsetOnAxis(ap=ti_all[:, j:j + 1], axis=0))
        nc.gpsimd.indirect_dma_start(
            out=gw[:], out_offset=None, in_=t["wpe_t"][:],
            in_offset=bass.IndirectOffsetOnAxis(ap=wi_all[:, j:j + 1], axis=0))
        cols = slice(j * 128, (j + 1) * 128)
        for c, (src, rows) in enumerate([(ge[:, 0:128], 128), (ge[:, 128:256], 128),
                                         (ge[:, 256:300], 44), (gw[:, :], 100)]):
            pt = pst.tile([128, 128], BF16, tag="pst")
            nc.tensor.transpose(out=pt[:rows, :], in_=src, identity=ident[:])
            dst = xT_pos[:rows, cols] if c == 3 else xT_emb[:rows, c, cols]
            if c % 2 == 0:
                nc.vector.tensor_copy(out=dst, in_=pt[:rows, :])
            else:
                nc.scalar.copy(out=dst, in_=pt[:rows, :])

    gpre_sb = {}

    def get_gpre(blk):
        if blk not in gpre_sb:
            gpre_sb[blk] = gp.tile([128, 2, 8, TBLK * 64], BF16, tag="gpre",
                                   name=f"gpre_b{blk}")
        return gpre_sb[blk]

    def emit_gpre(blk, d, m):
        gpre = get_gpre(blk)
        tok0 = blk * TBLK * 64 if d == 0 else (L - (blk + 1) * TBLK) * 64
        pgp = ps.tile([128, 512], F32, tag="ps")
        for k in range(4):
            rhs = xT_pos[:, tok0:tok0 + 512] if k == 3 else \
                xT_emb[:, k, tok0:tok0 + 512]
            nc.tensor.matmul(
                out=pgp[:], lhsT=wih_sb[:, d, k, m * 128:(m + 1) * 128],
                rhs=rhs, start=(k == 0), stop=(k == 3),
                skip_group_check=True)
        if m % 4 != 3:
            nc.vector.tensor_copy(out=gpre[:, d, m, :], in_=pgp[:])
        else:
            nc.scalar.copy(out=gpre[:, d, m, :], in_=pgp[:])

    # gpre free layout is [d:4096, m:512, col:1]; fwd reads col-offset tt*64,
    # bwd reads (TBLK-1-tt)*64 — fold both into one AP via an adjusted d-stride
    PP = 2 * 8 * TBLK * 64   # gpre partition pitch (free elems per partition)

    def gpre_ap(gpre, tt, nm):
        dstride = 8 * TBLK * 64 + (TBLK - 1 - 2 * tt) * 64
        return _ap(gpre[:, 0, 0, :], tt * 64,
                   [[PP, 128], [dstride, 2], [TBLK * 64, nm], [1, 64]])

    def emit_step(tau):
        blk, tt = tau // TBLK, tau % TBLK
        gpre = get_gpre(blk)
        sig = small.tile([128, 2, 8, 64], F32, tag="sig")
        tg = small.tile([128, 2, 2, 64], F32, tag="tg")
        if tau == 0:
            # no recurrent term: activations read g_pre from SBUF directly
            nc.scalar.activation(out=sig[:, :, 0:6, :], in_=gpre_ap(gpre, tt, 6),
                                 func=AF.Sigmoid)
            gsl = gpre_ap(gpre, tt, 8)
            tga = bass.AP(tensor=gsl.tensor, offset=gsl.offset + 6 * TBLK * 64,
                          ap=[[PP, 128], [gsl.ap[1][0], 2], [TBLK * 64, 2], [1, 64]])
            nc.scalar.activation(out=tg[:], in_=tga, func=AF.Tanh)
            gps = None
        if tau > 0:
            gps = psg.tile([128, 2, 8, 64], F32, tag="psg")
        m1 = small.tile([128, 2, 2, 64], F32, tag="m1")
        mm = small.tile([128, 2, 2, 64], F32, tag="mm")
        tc_ = small.tile([128, 2, 2, 64], F32, tag="tc_")
        # per-direction pipelining: d0's activations/elementwise chain runs
        # while d1's matmuls stream; the next step's d0 matmuls (which only
        # need h_d0) overlap d1's chain
        for d in range(2):
            if tau > 0:
                off = tt * 64 if d == 0 else (TBLK - 1 - tt) * 64
                # g_pre lands in PSUM via identity matmul: it has no h
                # dependency, so the PE runs it during the h(t-1) wait
                nc.tensor.matmul(
                    out=gps[:, d], lhsT=ident[:],
                    rhs=gpre[:, d, :, off:off + 64],
                    start=True, stop=False, skip_group_check=True)
                hcol = tau - 1 if d == 0 else L - tau
                for k in range(2):
                    for m in range(8):
                        nc.tensor.matmul(
                            out=gps[:, d, m],
                            lhsT=whh_sb[:, d, k, m * 128:(m + 1) * 128],
                            rhs=h_all[:, 2 * d + k, hcol * 64:(hcol + 1) * 64],
                            start=False, stop=(k == 1 and m == 7),
                            skip_group_check=True)
                nc.scalar.activation(out=sig[:, d, 0:6, :], in_=gps[:, d, 0:6, :],
                                     func=AF.Sigmoid)
                nc.scalar.activation(out=tg[:, d], in_=gps[:, d, 6:8, :],
                                     func=AF.Tanh)
            nc.vector.tensor_mul(out=m1[:, d], in0=sig[:, d, 0:2, :], in1=tg[:, d])
            if tau == 0:
                nc.vector.tensor_copy(out=c_st[:, d], in_=m1[:, d])
            else:
                nc.vector.tensor_mul(out=mm[:, d], in0=sig[:, d, 2:4, :],
                                     in1=c_st[:, d])
                nc.vector.tensor_add(out=c_st[:, d], in0=m1[:, d], in1=mm[:, d])
            nc.scalar.activation(out=tc_[:, d], in_=c_st[:, d], func=AF.Tanh)
            col = tau if d == 0 else L - 1 - tau
            nc.vector.tensor_mul(
                out=h_all[:, 2 * d:2 * d + 2, col * 64:(col + 1) * 64],
                in0=sig[:, d, 4:6, :], in1=tc_[:, d])

    # gather order per group g (fwd tiles first, then bwd tiles)
    def gorder(g):
        return [4 * g, 4 * g + 1, 4 * g + 2, 4 * g + 3,
                31 - 4 * g, 30 - 4 * g, 29 - 4 * g, 28 - 4 * g]

    # ---- word attention emission units (interleaved into late LSTM steps) ----
    uT_tiles = {}

    def emit_u_m(nb, m):
        if nb not in uT_tiles:
            uT_tiles[nb] = gp.tile([128, 5, 512], BF16, tag="uT",
                                   name=f"uT_nb{nb}")
        uT = uT_tiles[nb]
        cols = slice(nb * 512, (nb + 1) * 512)
        # pst pool is idle during steps 48-63; using it decouples the
        # u-block PSUM rotation from the g_pre unit rotation on "ps"
        pu = pst.tile([128, 512], F32, tag="pst")
        for k in range(5):
            rhs = xT_pos[:, cols] if k == 4 else h_all[:, k, cols]
            nc.tensor.matmul(
                out=pu[:], lhsT=wordW_sb[:, k, m * 128:(m + 1) * 128],
                rhs=rhs, start=(k == 0), stop=(k == 4))
        nc.scalar.activation(out=uT[:, m, :], in_=pu[:], func=AF.Tanh,
                             bias=wordb_sb[:, m:m + 1])

    def emit_u_score(nb):
        uT = uT_tiles[nb]
        psc = pst.tile([1, 512], F32, tag="pst")
        for k in range(5):
            nc.tensor.matmul(out=psc[:], lhsT=wproj_sb[:, k:k + 1], rhs=uT[:, k, :],
                            start=(k == 0), stop=(k == 4), skip_group_check=True)
        scst = small.tile([1, 512], F32, tag="scst")
        nc.vector.tensor_copy(out=scst[:], in_=psc[:])
        nc.sync.dma_start(out=t["scores_d"][nb * 512:(nb + 1) * 512], in_=scst[:])
        nc.sync.dma_start(out=sc2[0:64, 8 * nb:8 * nb + 8],
                          in_=_ap(t["scores_d"], nb * 512, [[1, 64], [64, 8]]))

    sc2 = st.tile([64, 64], F32)

    # flat unit queue for nb blocks whose h columns are complete mid-LSTM
    uq = []
    for nb in (3, 4, 2, 5, 1, 6):
        uq += [(nb, m) for m in range(5)] + [(nb, None)]

    # ---- pipelined word phase ----
    # fwd tiles first so d=0 g_pre can start after only 4 gathers
    g0 = gorder(0)
    for j in g0[:4]:
        emit_gather(j)
    for m in range(8):
        emit_gpre(0, 0, m)
    for j in g0[4:]:
        emit_gather(j)
    for m in range(8):
        emit_gpre(0, 1, m)
    uqi = 0
    for blk in range(NBLK):
        # groups 0-3 cover all 32 tiles (group g's bwd tiles = group 7-g's
        # fwd tiles), so only gather during blocks 0-2
        nxt = gorder(blk + 1) if blk + 1 <= 3 else None
        for tt in range(TBLK):
            tau = blk * TBLK + tt
            if nxt is not None and tt < 4:
                emit_gather(nxt[2 * tt])
                emit_gather(nxt[2 * tt + 1])
            if blk + 1 < NBLK and tt >= 4:
                i = tt - 4
                for q in range(4):
                    u = 4 * i + q
                    emit_gpre(blk + 1, u // 8, u % 8)
            emit_step(tau)
            # attention u-blocks for already-finished h columns
            if tau >= 48:
                budget = 2 if tau < 56 else 3
                for _ in range(budget):
                    if uqi < len(uq):
                        nb, m = uq[uqi]
                        uqi += 1
                        if m is None:
                            emit_u_score(nb)
                        else:
                            emit_u_m(nb, m)

    while uqi < len(uq):
        nb, m = uq[uqi]
        uqi += 1
        emit_u_score(nb) if m is None else emit_u_m(nb, m)
    for nb in (0, 7):
        for m in range(5):
            emit_u_m(nb, m)
        emit_u_score(nb)

    mx = small.tile([64, 1], F32, tag="mx")
    nc.vector.tensor_reduce(out=mx[:], in_=sc2[:], axis=AX.X, op=ALU.max, negate=True)
    ex = small.tile([64, 64], F32, tag="ex")
    den = small.tile([64, 1], F32, tag="den")
    nc.scalar.activation(out=ex[:], in_=sc2[:], func=AF.Exp, bias=mx[:],
                         accum_out=den[:])
    rcp = small.tile([64, 1], F32, tag="rcp")
    nc.vector.reciprocal(out=rcp[:], in_=den[:])
    alpha_bf = small.tile([64, 64], BF16, tag="alpha_bf")
    nc.vector.tensor_scalar_mul(out=alpha_bf[:], in0=ex[:], scalar1=rcp[:])
    alpha2 = small.tile([128, 64], BF16, tag="alpha2")
    nc.sync.dma_start(out=alpha2[0:64, :], in_=alpha_bf[:])
    nc.sync.dma_start(out=alpha2[64:128, :], in_=alpha_bf[:])
    # amat[p, j, s] = alpha[p%64, 2j + (p>=64)] * (p%64 == s)
    amat = amatp.tile([128, 32, 64], BF16, tag="amat")
    for half, eng in ((0, nc.vector), (1, nc.gpsimd)):
        pr = slice(half * 64, half * 64 + 64)
        eng.tensor_tensor(
            out=amat[pr, :, :],
            in0=_ap(m2_sb[pr, :], 0, [[64, 64], [0, 32], [1, 64]]),
            in1=_ap(alpha2[pr, :], half, [[64, 64], [2, 32], [0, 64]]),
            op=ALU.mult)
    hb_holder["t"] = big.tile([128, 32, 512], BF16, tag="xTe",
                              name="h_batch")
    h_batch = hb_holder["t"]
    for j in range(NTILE):
        emit_htile(j)
    psen = ps.tile([64, 512], F32, tag="ps")
    for j in range(NTILE):
        nc.tensor.matmul(out=psen[:], lhsT=amat[:, j, :], rhs=h_batch[:, j, :],
                        start=(j == 0), stop=(j == NTILE - 1),
                        skip_group_check=True)
    sen_sb = st.tile([64, 512], BF16)
    nc.vector.tensor_copy(out=sen_sb[:], in_=psen[:])

    # ---- sentence-level weights (reuse word slots) ----
    swih_sb = wgt.tile([128, 2, 5, 1024], BF16, tag="wih")
    swhh_sb = wgt.tile([128, 2, 2, 1024], BF16, tag="whh")
    for d in range(2):
        for k in range(5):
            nc.sync.dma_start(out=swih_sb[:, d, k, :], in_=t["swihT"][d, k])
        for k in range(2):
            nc.sync.dma_start(out=swhh_sb[:, d, k, :], in_=t["swhhT"][d, k])
    sentW_sb = wgt.tile([128, 5, 640], BF16, tag="attW")
    for k in range(5):
        nc.sync.dma_start(out=sentW_sb[:, k, :], in_=t["sentW"][k])
    sentb_sb = wgt.tile([128, 5], F32, tag="attb")
    nc.sync.dma_start(out=sentb_sb[:], in_=t["sentb"][:])
    sproj_sb = wgt.tile([128, 5], BF16, tag="attp")
    nc.sync.dma_start(out=sproj_sb[:], in_=t["sproj"][:])
    dW_sb = wgt.tile([128, 8, POL], BF16)
    for k in range(8):
        nc.sync.dma_start(out=dW_sb[:, k, :], in_=t["denseWT"][k])
    db_sb = wgt.tile([POL, 1], F32)
    nc.sync.dma_start(out=db_sb[:], in_=t["denseb"][:])

    # sxT [128, 5, 64] doc-major cols (d*16 + sigma)
    sxT = st.tile([128, 5, NS], BF16)
    nc.gpsimd.memset(sxT[:, 4, :], 0.0)
    nc.gpsimd.memset(sxT[96:128, 4, :], 1.0)
    si = gath.tile([NS, 1], I32, tag="ti")
    nc.sync.dma_start(out=si[:], in_=t["sp_idx"][:])
    gs = gath.tile([NS, P], BF16, tag="gw")
    nc.gpsimd.indirect_dma_start(
        out=gs[:], out_offset=None, in_=t["spe_t"][:],
        in_offset=bass.IndirectOffsetOnAxis(ap=si[:, :1], axis=0))
    pt_s = pst.tile([128, 64], BF16, tag="pst")
    nc.tensor.transpose(out=pt_s[:P, :], in_=gs[:], identity=ident[:64, :64])
    nc.vector.tensor_copy(out=sxT[:P, 4, :], in_=pt_s[:P, :])
    for c in range(4):
        ptv = pst.tile([128, 64], BF16, tag="pst")
        nc.tensor.transpose(out=ptv[:], in_=sen_sb[:, c * 128:(c + 1) * 128],
                            identity=ident[:64, :64])
        nc.vector.tensor_copy(out=sxT[:, c, :], in_=ptv[:])

    # sentence g_pre (all 16 steps at once)
    sgpre = st.tile([128, 2, 8, NS], BF16)
    for d in range(2):
        for m in range(8):
            pgs = ps.tile([128, 64], F32, tag="ps")
            for k in range(5):
                nc.tensor.matmul(
                    out=pgs[:], lhsT=swih_sb[:, d, k, m * 128:(m + 1) * 128],
                    rhs=sxT[:, k, :], start=(k == 0), stop=(k == 4))
            if m % 4 != 3:
                nc.vector.tensor_copy(out=sgpre[:, d, m, :], in_=pgs[:])
            else:
                nc.scalar.copy(out=sgpre[:, d, m, :], in_=pgs[:])

    # sentence BiLSTM (T=16, batch=4/dir), doc-major cols d*16+pos
    # tau=0: activations read sgpre directly; tau>0: whh matmul into PSUM,
    # then vector-add of the sgpre slice (no identity-extract matmul).
    hs_all = st.tile([128, 4, NS], BF16)
    cs_st = st.tile([128, 2, 2, ND], F32)
    for tau in range(S):
        if tau == 0:
            pos_f, pos_b = 0, S - 1
            src_f = _ap(sgpre[:, 0, 0, :], pos_f, [[1024, 128], [64, 8], [16, ND]])
            src_b = _ap(sgpre[:, 1, 0, :], pos_b, [[1024, 128], [64, 8], [16, ND]])
            sig = small.tile([128, 2, 8, ND], F32, tag="ssig")
            tg = small.tile([128, 2, 2, ND], F32, tag="stg")
            nc.scalar.activation(
                out=sig[:, 0, 0:6, :], func=AF.Sigmoid,
                in_=_ap(sgpre[:, 0, 0, :], pos_f, [[1024, 128], [64, 6], [16, ND]]))
            nc.scalar.activation(
                out=sig[:, 1, 0:6, :], func=AF.Sigmoid,
                in_=_ap(sgpre[:, 1, 0, :], pos_b, [[1024, 128], [64, 6], [16, ND]]))
            nc.scalar.activation(
                out=tg[:, 0], func=AF.Tanh,
                in_=_ap(sgpre[:, 0, 6, :], pos_f, [[1024, 128], [64, 2], [16, ND]]))
            nc.scalar.activation(
                out=tg[:, 1], func=AF.Tanh,
                in_=_ap(sgpre[:, 1, 6, :], pos_b, [[1024, 128], [64, 2], [16, ND]]))
        else:
            gps = psg.tile([128, 2, 8, 64], F32, tag="psg")
            sig = small.tile([128, 2, 8, ND], F32, tag="ssig")
            tg = small.tile([128, 2, 2, ND], F32, tag="stg")
        m1 = small.tile([128, 2, 2, ND], F32, tag="sm1")
        mm = small.tile([128, 2, 2, ND], F32, tag="smm")
        tc_ = small.tile([128, 2, 2, ND], F32, tag="stc")
        for d in range(2):
            if tau > 0:
                pos = tau if d == 0 else S - 1 - tau
                nc.tensor.matmul(
                    out=_ap(gps[:, d, 0, :], 0, [[1024, 128], [64, 8], [1, ND]]),
                    lhsT=ident[:],
                    rhs=_ap(sgpre[:, d, 0, :], pos, [[1024, 128], [64, 8], [16, ND]]),
                    start=True, stop=False, skip_group_check=True)
                hpos = tau - 1 if d == 0 else S - tau
                for k in range(2):
                    for m in range(8):
                        nc.tensor.matmul(
                            out=gps[:, d, m, 0:ND],
                            lhsT=swhh_sb[:, d, k, m * 128:(m + 1) * 128],
                            rhs=_ap(hs_all[:, 2 * d + k, :], hpos,
                                    [[256, 128], [16, ND]]),
                            start=False, stop=(k == 1 and m == 7),
                            skip_group_check=True)
                nc.scalar.activation(out=sig[:, d, 0:6, :],
                                     in_=gps[:, d, 0:6, 0:ND], func=AF.Sigmoid)
                nc.scalar.activation(out=tg[:, d], in_=gps[:, d, 6:8, 0:ND],
                                     func=AF.Tanh)
            nc.vector.tensor_mul(out=m1[:, d], in0=sig[:, d, 0:2, :], in1=tg[:, d])
            if tau == 0:
                nc.vector.tensor_copy(out=cs_st[:, d], in_=m1[:, d])
            else:
                nc.vector.tensor_mul(out=mm[:, d], in0=sig[:, d, 2:4, :],
                                     in1=cs_st[:, d])
                nc.vector.tensor_add(out=cs_st[:, d], in0=m1[:, d], in1=mm[:, d])
            nc.scalar.activation(out=tc_[:, d], in_=cs_st[:, d], func=AF.Tanh)
            pos = tau if d == 0 else S - 1 - tau
            nc.vector.tensor_mul(
                out=_ap(hs_all[:, 2 * d, :], pos, [[256, 128], [64, 2], [16, ND]]),
                in0=sig[:, d, 4:6, :], in1=tc_[:, d])

    # ---- sentence attention ----
    usT = st.tile([128, 5, NS], BF16)
    for m in range(5):
        pu = ps.tile([128, 64], F32, tag="ps")
        for k in range(5):
            rhs = sxT[:, 4, :] if k == 4 else hs_all[:, k, :]
            nc.tensor.matmul(out=pu[:], lhsT=sentW_sb[:, k, m * 128:(m + 1) * 128],
                            rhs=rhs, start=(k == 0), stop=(k == 4))
        nc.scalar.activation(out=usT[:, m, :], in_=pu[:], func=AF.Tanh,
                             bias=sentb_sb[:, m:m + 1])
    pscs = pst.tile([1, NS], F32, tag="pst")
    for k in range(5):
        nc.tensor.matmul(out=pscs[:], lhsT=sproj_sb[:, k:k + 1], rhs=usT[:, k, :],
                        start=(k == 0), stop=(k == 4))
    scs = small.tile([1, NS], F32, tag="scs")
    nc.vector.tensor_copy(out=scs[:], in_=pscs[:])
    sc2s = small.tile([ND, S], F32, tag="sc2s")
    nc.sync.dma_start(out=sc2s[:], in_=scs[0:1, 0:NS])
    mxs = small.tile([ND, 1], F32, tag="mxs")
    nc.vector.tensor_reduce(out=mxs[:], in_=sc2s[:], axis=AX.X, op=ALU.max,
                            negate=True)
    exs = small.tile([ND, S], F32, tag="exs")
    dens = small.tile([ND, 1], F32, tag="dens")
    nc.scalar.activation(out=exs[:], in_=sc2s[:], func=AF.Exp, bias=mxs[:],
                         accum_out=dens[:])
    rcs = small.tile([ND, 1], F32, tag="rcs")
    nc.vector.reciprocal(out=rcs[:], in_=dens[:])
    alps_bf = small.tile([ND, S], BF16, tag="alps_bf")
    nc.vector.tensor_scalar_mul(out=alps_bf[:], in0=exs[:], scalar1=rcs[:])
    # amats [64, 4]: column d rows d*16:(d+1)*16 = alps_bf[d, :]
    amats = small.tile([64, ND], BF16, tag="amats")
    nc.vector.memset(amats[:], 0.0)
    for d in range(ND):
        eng = nc.sync if d % 2 == 0 else nc.scalar
        eng.dma_start(out=amats[d * S:(d + 1) * S, d:d + 1],
                      in_=alps_bf[d:d + 1, :])
    hbs = st.tile([64, 512], BF16)
    for c in range(4):
        ptb = pst.tile([64, 128], BF16, tag="pst")
        nc.tensor.transpose(out=ptb[:], in_=hs_all[:, c, :], identity=ident[:])
        nc.vector.tensor_copy(out=hbs[:, c * 128:(c + 1) * 128], in_=ptb[:])
    # feats^T [128, 8, ND]: doc_vec computed directly in transposed layout
    featsT = st.tile([128, 8, ND], BF16)
    pdt = ps.tile([128, 4, ND], F32, tag="ps")
    for c in range(4):
        nc.tensor.matmul(out=pdt[:, c, :], lhsT=hbs[:, c * 128:(c + 1) * 128],
                         rhs=amats[:], start=(c == 0), stop=(c == 3),
                         skip_group_check=True)
    nc.vector.tensor_copy(out=featsT[:, 0:4, :], in_=pdt[:])
    nc.vector.tensor_copy(
        out=featsT[:, 4:6, :],
        in_=_ap(hs_all[:, 0, :], S - 1, [[256, 128], [64, 2], [16, ND]]))
    nc.vector.tensor_copy(
        out=featsT[:, 6:8, :],
        in_=_ap(hs_all[:, 2, :], 0, [[256, 128], [64, 2], [16, ND]]))
    pout = pst.tile([POL, ND], F32, tag="pst")
    for k in range(8):
        nc.tensor.matmul(out=pout[:], lhsT=dW_sb[:, k, :], rhs=featsT[:, k, :],
                        start=(k == 0), stop=(k == 7))
    yf = small.tile([POL, ND], F32, tag="yf")
    nc.scalar.activation(out=yf[:], in_=pout[:], func=AF.Identity, bias=db_sb[:])
    nc.sync.dma_start(out=_ap(t["y_out"][:], 0, [[1, POL], [POL, ND]]), in_=yf[:])


_NC_CACHE = None


def _get_nc():
    global _NC_CACHE
    if _NC_CACHE is None:
        _NC_CACHE = _build_nc()
    return _NC_CACHE


def _prep_host(inputs):
    gpm = GPERM
    wihs, whhs = [], []
    for wih, b in [(inputs["wWih_f"], inputs["wb_f"]),
                   (inputs["wWih_b"], inputs["wb_b"])]:
        wt = np.zeros((512, 1024), np.float32)
        wp = np.asarray(wih, np.float32)[gpm]      # [1024, 400]
        wt[0:300, :] = wp[:, 0:300].T
        wt[384:484, :] = wp[:, 300:400].T
        wt[511, :] = np.asarray(b, np.float32)[gpm]
        wihs.append(wt.reshape(4, 128, 1024))
    for whh in [inputs["wWhh_f"], inputs["wWhh_b"]]:
        whhs.append(np.asarray(whh, np.float32)[gpm].T.reshape(2, 128, 1024))
    swihs, swhhs = [], []
    for wih, b in [(inputs["sWih_f"], inputs["sb_f"]),
                   (inputs["sWih_b"], inputs["sb_b"])]:
        wt = np.zeros((640, 1024), np.float32)
        wp = np.asarray(wih, np.float32)[gpm]      # [1024, 612]
        wt[0:612, :] = wp.T
        wt[639, :] = np.asarray(b, np.float32)[gpm]
        swihs.append(wt.reshape(5, 128, 1024))
    for whh in [inputs["sWhh_f"], inputs["sWhh_b"]]:
        swhhs.append(np.asarray(whh, np.float32)[gpm].T.reshape(2, 128, 1024))

    def padW(w):
        o = np.zeros((640, 640), np.float32)
        o[:612, :612] = np.asarray(w, np.float32)
        return o.reshape(5, 128, 640)

    def padv(v, chunks):
        o = np.zeros((chunks * 128,), np.float32)
        o[:len(v)] = np.asarray(v, np.float32)
        return np.ascontiguousarray(o.reshape(chunks, 128).T)

    m2 = np.zeros((128, 64), np.float32)
    for p in range(128):
        m2[p, p % 64] = 1.0

    shared = {
        "emb_t": _bf(inputs["emb"]),
        "wpe_t": _bf(inputs["wpos_emb"]),
        "spe_t": _bf(inputs["spos_emb"]),
        "wihT": _bf(np.stack(wihs)),
        "whhT": _bf(np.stack(whhs)),
        "swihT": _bf(np.stack(swihs)),
        "swhhT": _bf(np.stack(swhhs)),
        "wordW": _bf(padW(inputs["word_W"])),
        "wordb": padv(inputs["word_bias"], 5).astype(np.float32),
        "wproj": _bf(padv(inputs["word_proj"], 5)),
        "sentW": _bf(padW(inputs["sent_W"])),
        "sentb": padv(inputs["sent_bias"], 5).astype(np.float32),
        "sproj": _bf(padv(inputs["sent_proj"], 5)),
        "denseWT": _bf(np.ascontiguousarray(
            np.asarray(inputs["dense_W"], np.float32).T).reshape(8, 128, POL)),
        "denseb": np.asarray(inputs["dense_b"], np.float32).reshape(POL, 1),
        "id128": _bf(np.eye(128)),
        "m2": _bf(m2),
    }

    toks = np.asarray(inputs["text_raw_indices"], np.int64).reshape(B, S, L)
    wpos = np.asarray(inputs["word_position"], np.int64).reshape(B, S, L)
    spos = np.asarray(inputs["segment_position"], np.int64).reshape(B, S)
    in_maps = []
    for c in range(NCORES):
        tk = toks[c * ND:(c + 1) * ND].reshape(NS, L)   # [64 s, 64 t]
        wp = wpos[c * ND:(c + 1) * ND].reshape(NS, L)
        sp = spos[c * ND:(c + 1) * ND]                  # [4, 16]
        m = dict(shared)
        m["tok_idx"] = np.ascontiguousarray(tk.T).reshape(NTILE, 128, 1).astype(np.int32)
        m["wp_idx"] = np.ascontiguousarray(wp.T).reshape(NTILE, 128, 1).astype(np.int32)
        m["sp_idx"] = sp.reshape(NS, 1).astype(np.int32)
        in_maps.append(m)
    return in_maps


def kernel(**inputs):
    nc = _get_nc()
    in_maps = _prep_host(inputs)
    res = run_bass_kernel_spmd(nc, in_maps, list(range(NCORES)))
    out = np.concatenate([res.results[c]["y"] for c in range(NCORES)], axis=0)
    return out.astype(np.float32)
